# revision 56
# baseline (speedup 1.0000x reference)
"""Trainium2 Bass kernel for nn_CrossAttention (B=2, Nq=Nk=2048, H=8, Dh=64,
Dx=512, Dctx=768).

Sharding: (batch, head-pair) across 8 cores — core c = (batch c//4, head pair
c%4). Each core computes K/V/Q projections for its 2 heads only, full
attention over 2048x2048 for those heads, and a PARTIAL output projection
(rows hp*128:(hp+1)*128 of Wo). The host sums the 4 bf16 partials per batch
at gather time (row-parallel Wo).

Changes vs the 132.9us baseline (measured ~111-112us):
- QK runs with TRUE K=64 contraction, row-tiled: head A's stationary lives in
  PE rows 0:63 (tile_position (0,0)), head B's in rows 64:127 ((64,0)).  kt
  is one [128, NKV] tile (A dh on partitions 0:64, B on 64:128) straight out
  of the kproj psum — no zero-pad memsets, and the two heads' matmuls can
  overlap in the array whenever both score psum tiles are free.
- The softmax reciprocal no longer touches the scalar engine (the baseline
  burned ~11.5us of the bottleneck engine on a Ln/Exp table trick): the raw
  denominators are broadcast with two K=1 PE matmuls (selector tile picks
  den row 64 of the psA copy / row 32 of psB — contracting over ONLY that
  row so unwritten NaN garbage never multiplies 0), one vector-engine
  reciprocal_approx_fast [128,512] inverts the broadcast, two tensor_tensor
  mults normalize.  Scalar does exp only: 64 x [128,1024] at ~1.11us each.
- Every dma_start costs ~600-700ns of serial DIRECT2D issue on the Sync
  engine, and non-contiguous (chunk-jumping) DRAM access patterns halve DMA
  bandwidth, so all inputs are HOST-PREARRANGED into layouts where each
  transfer is one pure-2D contiguous copy: packed weights blob (wk|wq|wv|wo),
  ctx in four kv-quarter params, xt split q-chunk-0 / rest.  7 input triggers
  ordered by the critical path to the first EXP (w, ctx_q0, xt_q0, ...);
  the first QK pair needs only 1.9MB of it.  Output stores are one 3D-AP
  trigger per qc (per-m 2D for the last qc so the drain overlaps oproj).
- kproj runs per kv-quarter: quarters 0/1 in the score psum banks before
  attention starts, quarters 2/3 in the acc pool during early QK pairs.
- ~48 junk matmuls on the constant tile during the input-DMA window ramp the
  PE clock (it needs ~4us of continuous work to leave the mid p-state) so
  kproj/qproj/QK start warm.
- Output partials are stored bf16 in a [128, 4*2048] packed layout (row p,
  col m*2048+q = partial[m*128+p, q]), host sums in f32 (emulated end-to-end
  numerics: rel err 0.0074 vs 2e-2 budget).
- CAUTION from measurement: some emission orders (e.g. moving the last PV of
  a qc after the next qc's first QK, or interleaving the two heads' QK
  matmuls per chunk) consistently flipped whole runs into a ~1.2x slower
  clock state or head-of-line-blocked the tensor queue.  The orders here are
  the empirically fast ones; re-measure EXP slice durations (~1114ns good,
  ~1335ns bad) after any scheduling change.
"""

import sys

sys.path.insert(0, "/opt/trn_rl_repo")

import numpy as np
import ml_dtypes

import concourse.bacc as bacc
import concourse.mybir as mybir
import concourse.tile as tile
from concourse.bass_utils import run_bass_kernel_spmd
from contextlib import ExitStack

F32 = mybir.dt.float32
BF16 = mybir.dt.bfloat16
NP_BF16 = np.dtype(ml_dtypes.bfloat16)

B = 2
NQ = 2048
NKV = 2048
DX = 512
DC = 768
DI = 512
NH = 8
DH = 64
N_CORES = 8

KC_X = DX // 128  # 4 contraction chunks for x
KC_C = DC // 128  # 6 contraction chunks for context
MO = DI // 128  # 4 output-row chunks
NKC = NKV // 128  # 16 kv chunks
NQC = NQ // 512  # 4 q chunks of 512
NPAIR = NKC // 2  # 8 kv chunk-pairs per q chunk
SCALE = DH ** -0.5
DW_ALL = KC_C * 128 + KC_X * 128 + KC_C * 128 + DI  # wk|wq|wv|wo = 2560

_CACHE = {}


def _build_nc():
    nc = bacc.Bacc("TRN2", target_bir_lowering=False, debug=False, num_devices=N_CORES)

    # Inputs are HOST-PREARRANGED so every transfer is a pure 2D copy of
    # contiguous DRAM rows: one trigger each, maximal descriptor segments,
    # no chunk-jumping access patterns (those halve DMA bandwidth).
    ctx_q = [nc.declare_dram_parameter(f"ctx_q{g}", [128, KC_C * 512], BF16,
                                       isOutput=False) for g in range(4)]
    xt_q0 = nc.declare_dram_parameter("xt_q0", [128, KC_X * 512], BF16, isOutput=False)
    xt_rest = nc.declare_dram_parameter("xt_rest", [128, KC_X * 1536], BF16, isOutput=False)
    w_all = nc.declare_dram_parameter("w_all", [128, DW_ALL], BF16, isOutput=False)
    # ot[p, m*NQ + q] = partial_out[m*128 + p, q]
    ot = nc.declare_dram_parameter("ot", [128, MO * NQ], BF16, isOutput=True)
    ot3d = ot.rearrange("p (m n) -> p m n", n=NQ)

    with tile.TileContext(nc) as tc:
        with ExitStack() as ctx:
            # ---- SBUF pools ----
            const_p = ctx.enter_context(tc.tile_pool(name="const", bufs=1))
            w_p = ctx.enter_context(tc.tile_pool(name="weights", bufs=1))
            ctx_p = ctx.enter_context(tc.tile_pool(name="ctxt", bufs=1))
            xt_p = ctx.enter_context(tc.tile_pool(name="xt", bufs=1))
            kt_p = ctx.enter_context(tc.tile_pool(name="kt", bufs=1))
            qt_p = ctx.enter_context(tc.tile_pool(name="qt", bufs=1))
            vaug_p = ctx.enter_context(tc.tile_pool(name="vaug", bufs=1))
            p_p = ctx.enter_context(tc.tile_pool(name="pp", bufs=3))
            at_p = ctx.enter_context(tc.tile_pool(name="at", bufs=2))
            out_p = ctx.enter_context(tc.tile_pool(name="outsb", bufs=2))
            # ---- PSUM pools: 4 (scores) + 2 (attn) + 2 (acc) = 8 banks ----
            acc_ps = ctx.enter_context(tc.tile_pool(name="acc_ps", bufs=2, space="PSUM"))
            s_ps = ctx.enter_context(tc.tile_pool(name="s_ps", bufs=1, space="PSUM"))
            attn_ps = ctx.enter_context(tc.tile_pool(name="attn_ps", bufs=1, space="PSUM"))

            # Only Exp runs on the scalar engine now; pin its table set once.
            nc.scalar.add_instruction(
                mybir.InstLoadActFuncSet(
                    name=nc.get_next_instruction_name(),
                    act_func_set_id=6, ins=[], outs=[]))

            # ---- constants ----
            # e_tile: bcast selector for the RAW denominators. Both heads'
            # denominators land on psum row 64 now (both V blocks are
            # [64 V | ones]), so row 64 is all-ones.
            e_tile = const_p.tile([128, 128], BF16)
            nc.any.memset(e_tile[:], 0.0)
            nc.any.memset(e_tile[64:65, :], 1.0)

            # ---- DMA inputs: few triggers, ordered by the critical path to
            # the first EXP (weights, ctx kv-half 0, xt q-chunk 0), with the
            # rest streaming behind. ----
            # One SBUF tile PER TRANSFER: a multi-dim DMA write is tracked at
            # whole-tile granularity, so readers of a shared big tile would
            # falsely wait on every later transfer into it.
            # DMA descriptors are per (row x contiguous DRAM segment), so ctx
            # splits by kv-HALF (2KB segments) not quarters (1KB would double
            # the descriptor count the sync engine must generate).
            w_sb = w_p.tile([128, DW_ALL], BF16, name="w_sb")
            ctxq = [ctx_p.tile([128, KC_C * 512], BF16, tag=f"cq{g}",
                               name=f"cq{g}") for g in range(4)]
            xtq0 = xt_p.tile([128, KC_X * 512], BF16, tag="xq0", name="xq0")
            xtr = xt_p.tile([128, KC_X * 1536], BF16, tag="xtr", name="xtr")

            # ctx_b is kv-QUARTER-major (quarter g: cols [g*3072,(g+1)*3072),
            # chunk c at +c*512) and xt_b is q-block-major (q 0:512: cols
            # [0,2048) chunk-major 512 each; q 512:2048: cols [2048,8192)
            # chunk-major 1536 each), so every transfer writes a CONTIGUOUS
            # column interval of its tile — the tile dependency tracker is
            # interval-based, and a strided write would false-couple every
            # later reader to the last transfer.
            nc.sync.dma_start(w_sb[:], w_all[:, :])
            nc.sync.dma_start(ctxq[0][:], ctx_q[0][:, :])
            nc.sync.dma_start(xtq0[:], xt_q0[:, :])
            nc.sync.dma_start(ctxq[1][:], ctx_q[1][:, :])
            nc.sync.dma_start(ctxq[2][:], ctx_q[2][:, :])
            nc.sync.dma_start(ctxq[3][:], ctx_q[3][:, :])
            nc.sync.dma_start(xtr[:], xt_rest[:, :])

            wk_t = [w_sb[:, c * 128:(c + 1) * 128] for c in range(KC_C)]
            _o = KC_C * 128
            wq_t = [w_sb[:, _o + c * 128:_o + (c + 1) * 128] for c in range(KC_X)]
            _o += KC_X * 128
            wv_t = [w_sb[:, _o + c * 128:_o + (c + 1) * 128] for c in range(KC_C)]
            _o += KC_C * 128
            wo_t = [w_sb[:, _o + m * 128:_o + (m + 1) * 128] for m in range(MO)]

            def ctx_c(c, lo, ln):
                # ctx kv cols [lo, lo+ln) of chunk c; must stay in one quarter
                g = lo // 512
                off = c * 512 + (lo - g * 512)
                return ctxq[g][:, off:off + ln]

            def xt_c(c, lo, ln):
                # x q cols [lo, lo+ln) of chunk c; q0 block or the rest
                if lo < 512:
                    return xtq0[:, c * 512 + lo:c * 512 + lo + ln]
                off = c * 1536 + (lo - 512)
                return xtr[:, off:off + ln]

            # ---- persistent activation tiles ----
            kt = kt_p.tile([128, NKV], BF16, name="kt")
            qt = qt_p.tile([128, NQ], BF16, name="qt")
            # vaug: per-kv-chunk layout [65 A | 65 B], both [64 V | ones]:
            # each head's attn lands on psum partitions 0:64 with its
            # denominator on row 64. B's attn is DMA-shifted to partitions
            # 64:128 after the psum evac (a 65-wide stationary matmul is
            # ~67ns cheaper than the old 128-wide B block, x64 PV matmuls).
            WC = 130  # 65 + 65 per chunk
            va = vaug_p.tile([128, NKC * WC + 65], BF16, name="va")
            va3 = va[:, 0:NKC * WC].rearrange("p (g c) -> p g c", c=WC)
            nc.any.memset(va3[:, :, 64:65], 1.0)    # A ones col
            nc.any.memset(va3[:, :, 129:130], 1.0)  # B ones col

            # ---- PE warm-up: the tensor clock ramps to full speed only
            # after ~4us of continuous matmuls; junk matmuls on the constant
            # tile fill the input-DMA window so kproj/qproj/QK start warm ----
            wu = acc_ps.tile([128, 128], F32, tag="acc", name="warmup")
            for _ in range(48):
                nc.tensor.matmul(wu[:], e_tile[:], e_tile[:],
                                 start=True, stop=True)

            # ---- K projection: kv-half 0 in the score psum banks before
            # attention starts, kv-half 1 in the acc pool during early QK
            # pairs (only gates pair 4+). ----
            def emit_kproj_h0():
                kp = s_ps.tile([128, 1024], F32, tag="s0", name="kp01")
                for g in range(2):
                    for c in range(KC_C):
                        nc.tensor.matmul(
                            kp[:, g * 512:(g + 1) * 512], wk_t[c],
                            ctx_c(c, g * 512, 512),
                            start=(c == 0), stop=(c == KC_C - 1))
                for g in range(2):
                    gs = slice(g * 512, (g + 1) * 512)
                    nc.vector.tensor_copy(kt[:, gs], kp[:, gs])

            def emit_kproj_h1():
                for g in range(2, 4):
                    ps = acc_ps.tile([128, 512], F32, tag="acc", name=f"kp{g}")
                    for c in range(KC_C):
                        nc.tensor.matmul(
                            ps[:], wk_t[c], ctx_c(c, g * 512, 512),
                            start=(c == 0), stop=(c == KC_C - 1))
                    nc.vector.tensor_copy(kt[:, g * 512:(g + 1) * 512], ps[:])

            def emit_qproj(n):
                ps = acc_ps.tile([128, 512], F32, tag="acc", name=f"pq{n}")
                for c in range(KC_X):
                    nc.tensor.matmul(
                        ps[:], wq_t[c], xt_c(c, n * 512, 512),
                        start=(c == 0), stop=(c == KC_X - 1))
                nc.vector.tensor_copy(qt[:, n * 512:(n + 1) * 512], ps[:])

            # ---- V projection, 4 kv chunks per psum tile; two strided
            # copies evacuate all 4 chunks x both heads ----
            def emit_v4(g):
                ps = acc_ps.tile([128, 512], F32, tag="acc", name=f"pv{g}")
                for j in range(4):
                    kvc = g * 4 + j
                    for c in range(KC_C):
                        nc.tensor.matmul(
                            ps[:, j * 128:(j + 1) * 128],
                            ctx_c(c, kvc * 128, 128), wv_t[c],
                            start=(c == 0), stop=(c == KC_C - 1))
                dst = va[:, g * 4 * WC:(g + 1) * 4 * WC].rearrange(
                    "p (c r) -> p c r", r=WC)
                src = ps[:].rearrange("p (c r) -> p c r", r=128)
                nc.vector.tensor_copy(dst[:, :, 0:64], src[:, :, 0:64])
                nc.vector.tensor_copy(dst[:, :, 65:129], src[:, :, 64:128])

            # ---- attention ----
            psa = {}  # (head) -> live attn psum tile
            psa_sb = {}  # (head) -> SBUF copy of attn + denom
            p_ts = {}  # (head, pair) -> P tile
            at_tiles = [None] * NQC

            def emit_qk(h, qc, p):
                # True K=64, row-tiled: head A in PE rows 0:63, head B in
                # 64:127 — base_partition auto-derives tile_position; the two
                # heads' matmuls run concurrently in the array.
                hs = slice(h * 64, (h + 1) * 64)
                ps_s = s_ps.tile([128, 1024], F32, tag=f"s{h}", name=f"s{h}_{qc}_{p}")
                for j in range(2):
                    kvc = p * 2 + j
                    nc.tensor.matmul(
                        ps_s[:, j * 512:(j + 1) * 512],
                        kt[hs, kvc * 128:(kvc + 1) * 128],
                        qt[hs, qc * 512:(qc + 1) * 512],
                        start=True, stop=True)
                p_t = p_p.tile([128, 1024], BF16, tag=f"p{h}", name=f"p{h}_{qc}_{p}")
                nc.scalar.activation(p_t[:], ps_s[:],
                                     mybir.ActivationFunctionType.Exp, scale=SCALE)
                p_ts[(h, p)] = p_t

            def emit_pv(h, qc, p):
                off = 65 * h
                if p == 0:
                    psa[h] = attn_ps.tile([65, 512], F32, tag=f"a{h}",
                                          name=f"a{h}_{qc}")
                for j in range(2):
                    kvc = p * 2 + j
                    nc.tensor.matmul(
                        psa[h][:], va[:, kvc * WC + off:kvc * WC + off + 65],
                        p_ts[(h, p)][:, j * 512:(j + 1) * 512],
                        start=(kvc == 0), stop=(kvc == NKC - 1))

            # normalize pipeline, spread across the next qc's pair iters.
            def emit_psevac(qc):
                tA = at_p.tile([65, 512], BF16, tag="psA", name=f"psA{qc}")
                nc.vector.tensor_copy(tA[:], psa[0][:])
                psa_sb[0] = tA
                # B evacs to partitions 0:65, then its attn rows are
                # DMA-shifted to partitions 64:128 (the only engine that can
                # move data across partitions) so the normalize multiply and
                # oproj moving stay base-aligned.
                tB = at_p.tile([65, 512], BF16, tag="psB", name=f"psB{qc}")
                nc.vector.tensor_copy(tB[:], psa[1][:])
                psa_sb[1] = tB
                tBs = at_p.tile([128, 512], BF16, tag="psBs", name=f"psBs{qc}")
                nc.sync.dma_start(tBs[64:128, :], tB[0:64, :])
                psa_sb[2] = tBs

            def emit_bcast(qc):
                # ps_b rows 0:64 <- d_A (tA row 64), rows 64:128 <- d_B (tB
                # row 64). Two K=1 matmuls contracting over ONLY the den row
                # — garbage rows in tA/tB are never touched (NaN * 0.0 would
                # poison a full-K contraction).
                ps_b = acc_ps.tile([128, 512], F32, tag="acc", name=f"bc{qc}")
                nc.tensor.matmul(ps_b[0:64, :], e_tile[64:65, 0:64],
                                 psa_sb[0][64:65, :], start=True, stop=True)
                nc.tensor.matmul(ps_b[64:128, :], e_tile[64:65, 64:128],
                                 psa_sb[1][64:65, :], start=True, stop=True)
                return ps_b

            def emit_recip(qc, ps_b):
                bc_sb = at_p.tile([128, 512], F32, tag="bc", name=f"bcs{qc}")
                nc.vector.reciprocal_approx_fast(out=bc_sb[:], in_=ps_b[:])
                return bc_sb

            def emit_atmult(qc, bc_sb):
                a_t = at_p.tile([128, 512], BF16, tag="at", name=f"at{qc}")
                nc.vector.tensor_tensor(a_t[0:64, :], psa_sb[0][0:64, :],
                                        bc_sb[0:64, :], op=mybir.AluOpType.mult)
                nc.vector.tensor_tensor(a_t[64:128, :], psa_sb[2][64:128, :],
                                        bc_sb[64:128, :], op=mybir.AluOpType.mult)
                at_tiles[qc] = a_t

            def emit_oproj(qc):
                # mid-flight qcs: one store trigger for all 4 output-row
                # chunks (sync DIRECT2D issue is ~600ns each). Last qc: per-m
                # stores so the drain overlaps the remaining oproj matmuls.
                split = qc == NQC - 1
                o_sb = out_p.tile([128, MO * 512], BF16, tag="osb", name=f"ob{qc}")
                for m in range(MO):
                    ps = acc_ps.tile([128, 512], F32, tag="acc", name=f"o{qc}_{m}")
                    nc.tensor.matmul(ps[:], wo_t[m], at_tiles[qc][:],
                                     start=True, stop=True)
                    nc.vector.tensor_copy(o_sb[:, m * 512:(m + 1) * 512], ps[:])
                    if split:
                        nc.sync.dma_start(
                            ot[:, m * NQ + qc * 512:m * NQ + (qc + 1) * 512],
                            o_sb[:, m * 512:(m + 1) * 512])
                if not split:
                    nc.sync.dma_start(
                        ot3d[:, :, qc * 512:(qc + 1) * 512],
                        o_sb[:].rearrange("p (m n) -> p m n", n=512))

            emit_kproj_h0()
            emit_qproj(0)
            pend_bc = None
            pend_rec = None
            for qc in range(NQC):
                for p in range(NPAIR + 1):
                    if p < NPAIR:
                        emit_qk(0, qc, p)
                        emit_qk(1, qc, p)
                    if qc == 0:
                        if p == 0:
                            emit_v4(0)
                        elif p == 1:
                            emit_kproj_h1()
                        elif p == 2:
                            emit_v4(1)
                        elif p == 3:
                            emit_qproj(1)
                        elif p == 4:
                            emit_v4(2)
                        elif p == 5:
                            emit_v4(3)
                    if qc in (1, 2) and p == 5:
                        emit_qproj(qc + 1)
                    if qc > 0:
                        # spread the previous qc's normalize chain so no
                        # single engine sees a block of serial work
                        if p == 0:
                            emit_psevac(qc - 1)
                        elif p == 1:
                            pend_bc = emit_bcast(qc - 1)
                        elif p == 2:
                            pend_rec = emit_recip(qc - 1, pend_bc)
                        elif p == 3:
                            emit_atmult(qc - 1, pend_rec)
                        elif p == 4:
                            emit_oproj(qc - 1)
                    if p >= 1:
                        emit_pv(0, qc, p - 1)
                        emit_pv(1, qc, p - 1)
            emit_psevac(NQC - 1)
            pend_bc = emit_bcast(NQC - 1)
            pend_rec = emit_recip(NQC - 1, pend_bc)
            emit_atmult(NQC - 1, pend_rec)
            emit_oproj(NQC - 1)

    nc.finalize()
    return nc


def _bf16(a):
    return np.ascontiguousarray(a).astype(NP_BF16)


def run_spmd(inputs, trace=False):
    if "nc" not in _CACHE:
        _CACHE["nc"] = _build_nc()
    nc = _CACHE["nc"]

    x = np.asarray(inputs["x"], dtype=np.float32)
    context = np.asarray(inputs["context"], dtype=np.float32)
    wq_f = np.asarray(inputs["Wq"], np.float32)
    wk_f = np.asarray(inputs["Wk"], np.float32)
    wv_f = np.asarray(inputs["Wv"], np.float32)
    wo_f = np.asarray(inputs["Wo"], np.float32)
    bo_f = np.asarray(inputs["bo"], np.float32)

    def pack(w):
        # [K*128, 128] -> [128, K*128]: row p holds chunk-c columns side by
        # side, so one 128-row DMA carries all contraction chunks
        k = w.shape[0] // 128
        return w.reshape(k, 128, 128).transpose(1, 0, 2).reshape(128, k * 128)

    def chunk_cols(aT, lo, hi):
        # [K*128, N] -> [128, K*(hi-lo)]: chunk-c cols lo:hi side by side
        k = aT.shape[0] // 128
        return _bf16(aT.reshape(k, 128, -1)[:, :, lo:hi]
                     .transpose(1, 0, 2).reshape(128, k * (hi - lo)))

    xT = [x[b].T for b in range(B)]
    cT = [context[b].T for b in range(B)]
    in_maps = []
    for c in range(N_CORES):
        b, hp = c // 4, c % 4
        cs = slice(hp * 128, (hp + 1) * 128)
        w_parts = np.concatenate(
            [pack(wk_f[:, cs]), pack(wq_f[:, cs]), pack(wv_f[:, cs]),
             wo_f[cs, :]], axis=1)
        in_maps.append({
            "ctx_q0": chunk_cols(cT[b], 0, 512),
            "ctx_q1": chunk_cols(cT[b], 512, 1024),
            "ctx_q2": chunk_cols(cT[b], 1024, 1536),
            "ctx_q3": chunk_cols(cT[b], 1536, 2048),
            "xt_q0": chunk_cols(xT[b], 0, 512),
            "xt_rest": chunk_cols(xT[b], 512, 2048),
            "w_all": _bf16(w_parts),
        })

    res = run_bass_kernel_spmd(nc, in_maps, core_ids=list(range(N_CORES)),
                               trace=trace)
    out = np.empty((B, NQ, DI), dtype=np.float32)
    for b in range(B):
        acc = res.results[b * 4]["ot"].astype(np.float32)
        for hp in range(1, 4):
            acc = acc + res.results[b * 4 + hp]["ot"].astype(np.float32)
        # ot[p, m*NQ + q] = partial[m*128 + p, q] -> out[q, di=m*128+p]
        out[b] = acc.reshape(128, MO, NQ).transpose(2, 1, 0).reshape(NQ, DI)
        out[b] += bo_f[None, :]
    return out, res


def kernel(**inputs):
    out, _ = run_spmd(inputs, trace=False)
    return out
